# revision 15
# baseline (speedup 1.0000x reference)
"""Trainium2 Bass kernel for nn_Block_89361089561275 (dense transformer block).

Sharding: data-parallel over batch B=8 -> one batch element per NeuronCore.
No collectives. Feature-transposed layout (features on SBUF partitions,
tokens on the free dim) so every matmul is a natural lhsT/rhs pair.

v2: fp8(e4m3) weights/activations with DoubleRow perf-mode matmuls (two
contraction chunks per instruction) for QKV/V/proj/MLP/adapter/AV; scores
stay bf16 (K=64 gets no DoubleRow benefit). rpb is passed log-domain in fp8
and folded into softmax via a single DVE (score/64 + rpb) pass, then one
scalar-engine exp -> fp8 P that feeds the AV DoubleRow matmul directly.
LN rstd = exp(-0.5*ln(var+eps)) so the only activation-table sets used are
natural_log_exp (LN1/attention/LN2) and gelu (MLP): 2 table loads total.
Weights are host-prescaled by 8 (fp8 subnormal avoidance); compensations are
folded into the gamma vectors, softmax scale, and activation scale operands.
All matmul accumulation is fp32 in PSUM; residual stream stays fp32.
"""

import sys

for _p in ("/opt/trn_rl_repo",):
    if _p not in sys.path:
        sys.path.insert(0, _p)

import numpy as np
import ml_dtypes

BF16 = ml_dtypes.bfloat16
F8 = ml_dtypes.float8_e4m3

B, N, C, H = 8, 1024, 768, 12
D = C // H            # 64
MLP = 4 * C           # 3072
RED = C // 3          # 256
EPS = 1e-5
P = 128
KC = C // P           # 6   c-chunks
KM = MLP // P         # 24  mlp-chunks
KR = RED // P         # 2   adapter chunks
NT = N // P           # 8   token tiles
HALF = 512
NSL = (slice(0, HALF), slice(HALF, N))
NP8 = N // P          # 8 scattered-stat free width

WS = 8.0              # host weight prescale (fp8 subnormal avoidance)

_PROG_CACHE: dict = {}

# indices into the packed [n, 128, KC] per-feature vector table
V_G1, V_G2, V_QB, V_PB, V_FB, V_L1G, V_L1B, V_L2G, V_L2B = range(9)
NVEC = 9


def _build(flags):
    """Build the single-core Bass program. flags is a tuple of bools:
    (has_mask, qb_nz, vb_nz, pb_nz, f1b_nz, fb_nz, adb_nz,
     ln1_gb_triv, ln2_gb_triv)
    """
    (has_mask, qb_nz, vb_nz, pb_nz, f1b_nz, fb_nz, adb_nz,
     ln1_triv, ln2_triv) = flags

    import concourse.tile as tile
    from concourse import bacc, mybir
    from contextlib import ExitStack

    f32 = mybir.dt.float32
    bf16 = mybir.dt.bfloat16
    f8 = mybir.dt.float8e4
    AF = mybir.ActivationFunctionType
    OP = mybir.AluOpType
    PM = mybir.MatmulPerfMode

    nc = bacc.Bacc("TRN2")

    # ---- external I/O ----
    x_d = nc.declare_dram_parameter("xT", [P, KC, N], f32, isOutput=False)
    rpb_d = nc.declare_dram_parameter("rpbT", [H, N, N], bf16, isOutput=False)
    wqk_d = nc.declare_dram_parameter("wqk", [12, P, KC, P], f8, isOutput=False)
    wv_d = nc.declare_dram_parameter("wv", [P, KC, C], f8, isOutput=False)
    pw_d = nc.declare_dram_parameter("projw", [KC, P, KC, P], f8, isOutput=False)
    f1_d = nc.declare_dram_parameter("fc1w", [KM, P, KC, P], f8, isOutput=False)
    f2_d = nc.declare_dram_parameter("fc2w", [KC, P, KM, P], f8, isOutput=False)
    ad_d = nc.declare_dram_parameter("adw", [KR, P, KC, P], f8, isOutput=False)
    au_d = nc.declare_dram_parameter("auw", [KC, P, KR, P], f8, isOutput=False)
    vec_d = nc.declare_dram_parameter("vecs", [NVEC, P, KC], f32, isOutput=False)
    f1b_d = nc.declare_dram_parameter("fc1b", [P, KM], f32, isOutput=False)
    adb_d = nc.declare_dram_parameter("adb", [P, KR], f32, isOutput=False)
    vb_d = nc.declare_dram_parameter("vbias", [1, C], f32, isOutput=False)
    mb_d = nc.declare_dram_parameter("maskb", [P, NT], f32, isOutput=False)
    out_d = nc.declare_dram_parameter("outT", [P, KC, N], f32, isOutput=True)

    with tile.TileContext(nc) as tc, ExitStack() as ctx:
        sb = ctx.enter_context(tc.tile_pool(name="sb", bufs=1))
        pp = ctx.enter_context(tc.tile_pool(name="pp", bufs=1, space="PSUM"))
        dram = ctx.enter_context(tc.tile_pool(name="dram", bufs=2, space="DRAM"))

        def broadcast(dst, src):
            """dst [p, n] sbuf <- src [1, n] sbuf/psum replicated across
            partitions (via a DRAM bounce)."""
            scratch = dram.tile([1, src.shape[-1]], src.dtype,
                                tag="bscratch", bufs=2, name="bscratch")
            nc.sync.dma_start(out=scratch, in_=src)
            nc.sync.dma_start(out=dst, in_=scratch.to_broadcast(dst.shape))

        def bcast_from_p128(dst, src128):
            """dst [p, n] sbuf <- broadcast of a [128, n/128] scattered row."""
            n = dst.shape[-1]
            scratch = dram.tile([1, n], src128.dtype,
                                tag="bscratch", bufs=2, name="bscratch")
            nc.sync.dma_start(out=scratch, in_=src128)
            nc.sync.dma_start(out=dst, in_=scratch.to_broadcast(dst.shape))

        # ---- persistent tiles ----
        xres = sb.tile([P, KC, N], f32, tag="xres", bufs=1)
        qkT = sb.tile([P, 12, N], bf16, tag="qkT", bufs=1)
        vaug = sb.tile([P, NT // 2, H, 2, D + 16], f8, tag="vaug", bufs=1)
        ones_bf = sb.tile([P, 1], bf16, tag="ones", bufs=1)

        for ch in range(KC):  # per-chunk loads so LN1 stats start early
            nc.sync.dma_start(out=xres[:, ch], in_=x_d[:, ch])
        nc.vector.memset(ones_bf, 1.0)
        nc.vector.memset(vaug, 1.0)

        zero_col = sb.tile([P, 1], f32, tag="zcol", bufs=1)
        nc.vector.memset(zero_col, 0.0)
        eps_col = sb.tile([P, 1], f32, tag="ecol", bufs=1)
        nc.vector.memset(eps_col, float(EPS))

        vecs = sb.tile([P, NVEC, KC], f32, tag="vecs", bufs=1)
        nc.sync.dma_start(out=vecs, in_=vec_d[:].rearrange("v p k -> p v k"))

        def vec(i):
            return vecs[:, i]  # [128, KC]

        if f1b_nz:
            f1b = sb.tile([P, KM], f32, tag="f1b", bufs=1)
            nc.sync.dma_start(out=f1b, in_=f1b_d[:])
        if adb_nz:
            adb = sb.tile([P, KR], f32, tag="adb", bufs=1)
            nc.sync.dma_start(out=adb, in_=adb_d[:])
        if vb_nz:
            vb1 = sb.tile([1, C], f32, tag="vb1", bufs=1)
            nc.sync.dma_start(out=vb1, in_=vb_d[:])
            vb_b = sb.tile([P, C], f32, tag="vb_b", bufs=1)
            broadcast(vb_b, vb1)
        if has_mask:
            maskb = sb.tile([P, NT], f32, tag="maskb", bufs=1)
            nc.sync.dma_start(out=maskb, in_=mb_d[:])

        # ---------------- layernorm (feature-transposed) ----------------
        # Stats via matmul: sum(x) on PSUM row 0, sum(x^2) on row 32.
        def layernorm(dst, g_i, b_i, triv):
            stt = pp.tile([33, N], f32, tag="st", bufs=2, name="ln_stats")
            xbs = []
            for ch in range(KC):
                xb = sb.tile([P, N], bf16, tag="xb", bufs=KC)
                xbs.append(xb)
                nc.vector.tensor_copy(out=xb, in_=xres[:, ch])
                x2 = sb.tile([P, N], bf16, tag="x2", bufs=2)
                nc.scalar.square(out=x2, in_=xres[:, ch])
                for nk in range(2):
                    nc.tensor.matmul(stt[0:1, NSL[nk]], lhsT=ones_bf,
                                     rhs=xb[:, NSL[nk]],
                                     start=(ch == 0), stop=(ch == KC - 1))
                    nc.tensor.matmul(stt[32:33, NSL[nk]], lhsT=ones_bf,
                                     rhs=x2[:, NSL[nk]],
                                     start=(ch == 0), stop=(ch == KC - 1))
            # PSUM rows -> SBUF (DMA cannot read PSUM), then scatter the
            # per-token stats across 128 partitions and finish there
            stat2 = sb.tile([33, N], f32, tag="stat", bufs=2)
            nc.vector.tensor_copy(out=stat2, in_=stt)
            m128 = sb.tile([P, NP8], f32, tag="cs128", bufs=4)
            q128 = sb.tile([P, NP8], f32, tag="cs128", bufs=4)
            nc.sync.dma_start(out=m128, in_=stat2[0:1, :])
            nc.sync.dma_start(out=q128, in_=stat2[32:33, :])
            t128 = sb.tile([P, NP8], f32, tag="cs128", bufs=4)
            r128 = sb.tile([P, NP8], f32, tag="cs128", bufs=4)
            nc.vector.tensor_scalar_mul(m128, m128, 1.0 / C)
            nc.vector.tensor_scalar_mul(q128, q128, 1.0 / C)
            nc.vector.tensor_mul(t128, m128, m128)
            nc.vector.tensor_sub(q128, q128, t128)       # var
            # rstd = exp(-0.5*ln(var+eps)): stays in the natural_log_exp
            # activation-table set (no sqrt-set load)
            nc.scalar.activation(t128, q128, AF.Ln, bias=eps_col)
            nc.scalar.activation(r128, t128, AF.Exp, scale=-0.5)
            # m128 <- -mean*rstd
            nc.vector.scalar_tensor_tensor(out=m128, in0=m128, scalar=-1.0,
                                           in1=r128, op0=OP.mult, op1=OP.mult)
            r_bf = sb.tile([P, NP8], bf16, tag="cs_bf", bufs=2)
            n_bf = sb.tile([P, NP8], bf16, tag="cs_bf", bufs=2)
            nc.vector.tensor_copy(out=r_bf, in_=r128)
            nc.vector.tensor_copy(out=n_bf, in_=m128)
            a_b = sb.tile([P, N], bf16, tag="bcast", bufs=2)
            c_b = sb.tile([P, N], bf16, tag="bcast", bufs=2)
            bcast_from_p128(a_b, r_bf)
            bcast_from_p128(c_b, n_bf)
            for ch in range(KC):
                t1 = sb.tile([P, N], bf16, tag="x2", bufs=2)
                nc.vector.tensor_mul(t1, xbs[ch], a_b)
                if triv:
                    nc.vector.tensor_add(dst[:, ch], t1, c_b)
                else:
                    nc.vector.tensor_add(t1, t1, c_b)
                    nc.vector.tensor_scalar(
                        out=dst[:, ch], in0=t1,
                        scalar1=vec(g_i)[:, ch:ch + 1],
                        scalar2=vec(b_i)[:, ch:ch + 1],
                        op0=OP.mult, op1=OP.add)

        # ---------------- LN1 + QKV ----------------
        xnT = sb.tile([P, KC, N], f8, tag="feat", bufs=2)
        layernorm(xnT, V_L1G, V_L1B, ln1_triv)
        # pair-contiguous per-token-tile copy so the v matmul can DoubleRow
        xnv = sb.tile([P, NT, KC // 2, 2, P], f8, tag="xnv", bufs=1)
        for ch in range(KC):
            nc.vector.tensor_copy(
                out=xnv[:, :, ch // 2, ch % 2, :],
                in_=xnT[:, ch].rearrange("p (t w) -> p t w", w=P))

        # q/k blocks interleaved so attention head-pair j can start as soon
        # as blocks j (q) and 6+j (k) are done
        for blk in (0, 6, 1, 7, 2, 8, 3, 9, 4, 10, 5, 11):
            wt = sb.tile([P, KC, P], f8, tag="w6", bufs=3)
            nc.sync.dma_start(out=wt, in_=wqk_d[blk])
            for nk in range(2):
                mm = pp.tile([P, HALF], f32, tag="acc", bufs=4)
                for kp in range(KC // 2):
                    nc.tensor.matmul(mm, lhsT=wt[:, 2 * kp:2 * kp + 2],
                                     rhs=xnT[:, 2 * kp:2 * kp + 2, NSL[nk]],
                                     start=(kp == 0), stop=(kp == KC // 2 - 1),
                                     perf_mode=PM.DoubleRow)
                dst = qkT[:, blk, NSL[nk]]
                if blk < 6 and qb_nz:
                    nc.vector.tensor_scalar_add(dst, mm, vec(V_QB)[:, blk:blk + 1])
                else:
                    nc.scalar.copy(out=dst, in_=mm)

        wv_sb = sb.tile([P, KC, C], f8, tag="wv", bufs=1)
        nc.sync.dma_start(out=wv_sb, in_=wv_d[:])
        for t in range(NT):
            for off, cw in ((0, HALF), (HALF, C - HALF)):
                mm = pp.tile([P, HALF], f32, tag="acc", bufs=4)
                for kp in range(KC // 2):
                    nc.tensor.matmul(
                        mm[:, :cw],
                        lhsT=xnv[:, t, kp],
                        rhs=wv_sb[:, 2 * kp:2 * kp + 2, off:off + cw],
                        start=(kp == 0), stop=(kp == KC // 2 - 1),
                        perf_mode=PM.DoubleRow)
                dst = vaug[:, t // 2, off // D:(off + cw) // D, t % 2, :D]
                src = mm[:, :cw].rearrange("p (h d) -> p h d", d=D)
                if vb_nz:
                    nc.vector.tensor_add(
                        dst, src,
                        vb_b[:, off:off + cw].rearrange("p (h d) -> p h d", d=D))
                else:
                    nc.scalar.copy(out=dst, in_=src)

        # ---------------- attention ----------------
        # Heads (2j, 2j+1) at partition offsets 0/64 of qkT block j; the two
        # K=64 score matmuls run in distinct PE row-groups. P~ = exp(s/64+rpb)
        # computed as one DVE pass (score scale + log-rpb add) + one scalar
        # exp straight to fp8, feeding the AV DoubleRow matmul (2 m-tiles per
        # instruction).
        oT = sb.tile([P, KC, N], f8, tag="feat", bufs=2)

        def evac_head(o_ps, hp, hh):
            # unnormalized head in PSUM [65, 512] x2; row 64 = colsum.
            # Copy to SBUF right away (frees the accumulator banks; DMA
            # cannot read PSUM; scalar engine has slack here), broadcast
            # the colsum, normalize async.
            ou = sb.tile([D + 16, N], f32, tag="stat", bufs=2)
            for nk in range(2):
                nc.scalar.copy(out=ou[:, NSL[nk]], in_=o_ps[nk])
            raw = sb.tile([P, N], f32, tag="rec", bufs=2)
            broadcast(raw[0:D, :], ou[D:D + 1, :])
            rb = sb.tile([P, N], f32, tag="rec2", bufs=2)
            nc.vector.reciprocal_approx_fast(out=rb[0:D, :], in_=raw[0:D, :])
            ot_tmp = sb.tile([P, N], f8, tag="ott", bufs=2)
            # 16/colsum * o_unnorm (the 16 re-centers fp8; undone in g1)
            nc.vector.scalar_tensor_tensor(
                out=ot_tmp[0:D, :], in0=ou[0:D, :], scalar=16.0,
                in1=rb[0:D, :], op0=OP.mult, op1=OP.mult)
            nc.sync.dma_start(out=oT[hh * D:(hh + 1) * D, hp, :],
                              in_=ot_tmp[0:D, :])

        for hp in range(H // 2):
            qh = [qkT[hh * D:(hh + 1) * D, hp, :] for hh in range(2)]
            kh = [qkT[hh * D:(hh + 1) * D, 6 + hp, :] for hh in range(2)]
            o_ps = [[pp.tile([D + 16, HALF], f32, tag="acc", bufs=4,
                             name=f"o_ps{hh}{nk}") for nk in range(2)]
                    for hh in range(2)]
            p8 = [sb.tile([P, 2, N], f8, tag="p8", bufs=4,
                          name=f"p8_{hh}") for hh in range(2)]
            for mt in range(NT):
                for hh in range(2):
                    h = 2 * hp + hh
                    rpb_t = sb.tile([P, N], bf16, tag="rpb", bufs=6)
                    nc.sync.dma_start(out=rpb_t,
                                      in_=rpb_d[h, mt * P:(mt + 1) * P, :])
                    st = pp.tile([P, N], f32, tag="st", bufs=2, name="st")
                    for nk in range(2):
                        nc.tensor.matmul(st[:, NSL[nk]],
                                         lhsT=kh[hh][:, mt * P:(mt + 1) * P],
                                         rhs=qh[hh][:, NSL[nk]],
                                         start=True, stop=True)
                    # P~ = exp(s/512)*exp(rpb): scalar reads PSUM with its
                    # free affine scale; DVE does an all-16-bit multiply
                    texp = sb.tile([P, N], bf16, tag="texp", bufs=3)
                    nc.scalar.activation(out=texp, in_=st, func=AF.Exp,
                                         scale=1.0 / 512.0)
                    pslot = p8[hh][:, mt % 2, :]
                    if has_mask:
                        nc.vector.scalar_tensor_tensor(
                            out=pslot, in0=texp, scalar=maskb[:, mt:mt + 1],
                            in1=rpb_t, op0=OP.mult, op1=OP.mult)
                    elif (hh == 0 and mt in (1, 3, 5)) or \
                            (hh == 1 and mt in (2, 4, 6)):
                        # offload ~6/16 of the P=exp(s)*exp(rpb) multiplies
                        # per head-pair to the otherwise-idle GpSimd engine
                        nc.gpsimd.tensor_tensor(out=pslot, in0=texp,
                                                in1=rpb_t, op=OP.mult)
                    else:
                        nc.vector.tensor_mul(pslot, texp, rpb_t)
                if mt % 2 == 1:
                    for hh in range(2):
                        for nk in range(2):
                            nc.tensor.matmul(
                                o_ps[hh][nk][:, :],
                                lhsT=vaug[:, mt // 2, 2 * hp + hh, :, :],
                                rhs=p8[hh][:, :, NSL[nk]],
                                start=(mt == 1), stop=(mt == NT - 1),
                                perf_mode=PM.DoubleRow)
                    if mt < NT - 1:
                        p8 = [sb.tile([P, 2, N], f8, tag="p8", bufs=4,
                                      name=f"p8_{hh}") for hh in range(2)]
            evac_head(o_ps[0], hp, 0)
            evac_head(o_ps[1], hp, 1)

        # ---------------- proj + residual 1 ----------------
        for mt in range(KC):
            wt = sb.tile([P, KC, P], f8, tag="w6", bufs=3)
            nc.sync.dma_start(out=wt, in_=pw_d[mt])
            for nk in range(2):
                mm = pp.tile([P, HALF], f32, tag="acc", bufs=4)
                for kp in range(KC // 2):
                    nc.tensor.matmul(mm, lhsT=wt[:, 2 * kp:2 * kp + 2],
                                     rhs=oT[:, 2 * kp:2 * kp + 2, NSL[nk]],
                                     start=(kp == 0), stop=(kp == KC // 2 - 1),
                                     perf_mode=PM.DoubleRow)
                if pb_nz:
                    nc.vector.tensor_scalar_add(mm, mm, vec(V_PB)[:, mt:mt + 1])
                nc.vector.scalar_tensor_tensor(
                    out=xres[:, mt, NSL[nk]], in0=mm,
                    scalar=vec(V_G1)[:, mt:mt + 1],
                    in1=xres[:, mt, NSL[nk]], op0=OP.mult, op1=OP.add)

        # ---------------- LN2, adapter-down, MLP ----------------
        xn2T = sb.tile([P, KC, N], f8, tag="feat", bufs=2)
        layernorm(xn2T, V_L2G, V_L2B, ln2_triv)

        a1T = sb.tile([P, KR, N], f8, tag="a1", bufs=1)
        for mt in range(KR):
            wt = sb.tile([P, KC, P], f8, tag="w6", bufs=3)
            nc.sync.dma_start(out=wt, in_=ad_d[mt])
            for nk in range(2):
                mm = pp.tile([P, HALF], f32, tag="acc", bufs=4)
                for kp in range(KC // 2):
                    nc.tensor.matmul(mm, lhsT=wt[:, 2 * kp:2 * kp + 2],
                                     rhs=xn2T[:, 2 * kp:2 * kp + 2, NSL[nk]],
                                     start=(kp == 0), stop=(kp == KC // 2 - 1),
                                     perf_mode=PM.DoubleRow)
                nc.scalar.activation(
                    out=a1T[:, mt, NSL[nk]], in_=mm, func=AF.Relu,
                    scale=1.0 / WS,
                    bias=(adb[:, mt:mt + 1] if adb_nz else zero_col))

        # fc1 for BOTH token halves interleaved: 12 DoubleRow matmuls per
        # gelu-pair window keeps the PE dense (max p-state); each fc1 weight
        # tile is loaded once. Two output blocks share one 2-bank PSUM tile
        # so a single [128,1024] gelu covers them.
        h1s = [sb.tile([P, KM, HALF], f8, tag=f"h1_{nk}", bufs=1,
                       name=f"h1_{nk}") for nk in range(2)]
        for mtp in range(KM // 2):
            wts = []
            for j in range(2):
                wt = sb.tile([P, KC, P], f8, tag="w6", bufs=3)
                nc.sync.dma_start(out=wt, in_=f1_d[2 * mtp + j])
                wts.append(wt)
            for nk in range(2):
                mm2 = pp.tile([P, N], f32, tag="st", bufs=2)
                for j in range(2):
                    for kp in range(KC // 2):
                        nc.tensor.matmul(mm2[:, j * HALF:(j + 1) * HALF],
                                         lhsT=wts[j][:, 2 * kp:2 * kp + 2],
                                         rhs=xn2T[:, 2 * kp:2 * kp + 2, NSL[nk]],
                                         start=(kp == 0),
                                         stop=(kp == KC // 2 - 1),
                                         perf_mode=PM.DoubleRow)
                h1 = h1s[nk]
                if f1b_nz:
                    for j in range(2):
                        mt = 2 * mtp + j
                        nc.scalar.activation(
                            out=h1[:, mt], in_=mm2[:, j * HALF:(j + 1) * HALF],
                            func=AF.Gelu, scale=1.0 / WS,
                            bias=f1b[:, mt:mt + 1])
                else:
                    nc.scalar.activation(
                        out=h1[:, 2 * mtp:2 * mtp + 2].rearrange(
                            "p a t -> p (a t)"),
                        in_=mm2, func=AF.Gelu, scale=1.0 / WS, bias=zero_col)
        for mt in range(KC):
            w2 = sb.tile([P, KM, P], f8, tag="w24", bufs=2)
            nc.sync.dma_start(out=w2, in_=f2_d[mt])
            au = sb.tile([P, KR, P], f8, tag="w2", bufs=2)
            nc.sync.dma_start(out=au, in_=au_d[mt])
            for nk in range(2):
                mm = pp.tile([P, HALF], f32, tag="acc", bufs=4)
                for kp in range(KM // 2):
                    nc.tensor.matmul(mm, lhsT=w2[:, 2 * kp:2 * kp + 2],
                                     rhs=h1s[nk][:, 2 * kp:2 * kp + 2],
                                     start=(kp == 0), stop=False,
                                     perf_mode=PM.DoubleRow)
                nc.tensor.matmul(mm, lhsT=au, rhs=a1T[:, :, NSL[nk]],
                                 start=False, stop=True,
                                 perf_mode=PM.DoubleRow)
                if fb_nz:
                    nc.vector.tensor_scalar_add(mm, mm, vec(V_FB)[:, mt:mt + 1])
                nc.vector.scalar_tensor_tensor(
                    out=xres[:, mt, NSL[nk]], in0=mm,
                    scalar=vec(V_G2)[:, mt:mt + 1],
                    in1=xres[:, mt, NSL[nk]], op0=OP.mult, op1=OP.add)
                nc.sync.dma_start(out=out_d[:, mt, NSL[nk]],
                                  in_=xres[:, mt, NSL[nk]])

    if not nc.is_finalized():
        nc.finalize()
    return nc


def _pack_w6(wT, km, kk):
    """[K, M] (K=contraction, M=out) -> [M//128, 128, K//128, 128] tiles
    laid out so each DMA partition read is contiguous."""
    K, M = wT.shape
    assert K == kk * P and M == km * P
    a = wT.reshape(kk, P, km, P)          # [ks, p, mt, col]
    return np.ascontiguousarray(a.transpose(2, 1, 0, 3)).astype(F8)


def _stripe(v, k):
    """[k*128] -> [128, k] with v[ks*128+p] at [p, ks]."""
    return np.ascontiguousarray(v.reshape(k, P).T).astype(np.float32)


def prepare_core_inputs(x, mask, rpb, ln1_g, ln1_b, qkv_w, q_bias, v_bias,
                        proj_w, proj_b, gamma1, ln2_g, ln2_b, fc1_w, fc1_b,
                        fc2_w, fc2_b, ad_dw, ad_db, ad_uw, ad_ub, gamma2):
    """Host-side layout prep. Returns (per_core_maps, flags)."""
    f32 = np.float32

    qkv_w = np.asarray(qkv_w, f32)
    wq = qkv_w[:C] * WS            # both x8; with 1/sqrt(D)=1/8 the scores
    wk = qkv_w[C:2 * C] * WS       # need a 1/(8*8*8)=1/512 softmax-side scale
    wv = qkv_w[2 * C:] * WS
    wqkT = np.concatenate([wq, wk], 0).T          # [C, 1536]
    wqk = _pack_w6(wqkT, 12, KC)
    # wv used as matmul rhs: [p, ks, col] = wv[col, ks*128+p]
    wv_packed = np.ascontiguousarray(
        wv.T.reshape(KC, P, C).transpose(1, 0, 2)).astype(F8)

    projw = _pack_w6(np.asarray(proj_w, f32).T * WS, KC, KC)
    fc1w = _pack_w6(np.asarray(fc1_w, f32).T * WS, KM, KC)
    fc2w = _pack_w6(np.asarray(fc2_w, f32).T * WS, KC, KM)
    adw = _pack_w6(np.asarray(ad_dw, f32).T * WS, KR, KC)
    auw = _pack_w6(np.asarray(ad_uw, f32).T * WS, KC, KR)

    # exp(rpb) so the kernel folds the bias into softmax as a multiply
    rpbT = np.ascontiguousarray(
        np.exp(np.asarray(rpb, f32).transpose(0, 2, 1))).astype(BF16)

    fb = (np.asarray(fc2_b, f32) + np.asarray(ad_ub, f32)) * WS

    # gamma1 compensates: v x8, evac x16, projw x8  -> /1024
    # gamma2 compensates: fc2w/auw x8 -> /8; fb prescaled x8 to match
    vecs = np.stack([
        _stripe(np.asarray(gamma1, f32) / (WS * 16.0 * WS), KC),
        _stripe(np.asarray(gamma2, f32) / WS, KC),
        _stripe(np.asarray(q_bias, f32) * WS, KC),
        _stripe(np.asarray(proj_b, f32) * (WS * 16.0 * WS), KC),
        _stripe(fb, KC),
        _stripe(np.asarray(ln1_g, f32), KC),
        _stripe(np.asarray(ln1_b, f32), KC),
        _stripe(np.asarray(ln2_g, f32), KC),
        _stripe(np.asarray(ln2_b, f32), KC),
    ], 0)  # [NVEC, 128, KC]

    f1b = _stripe(np.asarray(fc1_b, f32), KM)
    adb = _stripe(np.asarray(ad_db, f32), KR)
    vb = (np.asarray(v_bias, f32) * WS).reshape(1, C).astype(f32)

    mask = np.asarray(mask)
    has_mask = not bool(mask.all())

    flags = (
        has_mask,
        bool(np.any(np.asarray(q_bias, f32))),
        bool(np.any(v_bias)),
        bool(np.any(proj_b)),
        bool(np.any(fc1_b)),
        bool(np.any(fb)),
        bool(np.any(ad_db)),
        bool(np.all(ln1_g == 1.0) and not np.any(ln1_b)),
        bool(np.all(ln2_g == 1.0) and not np.any(ln2_b)),
    )

    shared = {
        "rpbT": rpbT, "wqk": wqk, "wv": wv_packed, "projw": projw,
        "fc1w": fc1w, "fc2w": fc2w, "adw": adw, "auw": auw,
        "vecs": vecs, "fc1b": f1b, "adb": adb, "vbias": vb,
    }

    x = np.asarray(x, f32)
    per_core = []
    for b in range(B):
        xT = np.ascontiguousarray(
            x[b].T.reshape(KC, P, N).transpose(1, 0, 2)).astype(f32)
        if has_mask:
            mb = np.where(mask[b], 1.0, 0.0).astype(f32)    # [N] over keys m
            mb = np.ascontiguousarray(mb.reshape(NT, P).T)  # [128, NT]
        else:
            mb = np.zeros((P, NT), f32)
        m = dict(shared)
        m["xT"] = xT
        m["maskb"] = mb
        per_core.append(m)
    return per_core, flags


def _ensure_ntff_hook():
    """The agent image lacks ``antenv.axon_hooks``; provide it and register
    the ctypes NTFF profile hook so trace=True works under axon."""
    import types
    try:
        from antenv.axon_hooks import get_axon_ntff_profile_hook  # noqa: F401
        return
    except ImportError:
        pass
    import antenv
    mod = types.ModuleType("antenv.axon_hooks")
    _h = {"hook": None}
    mod.set_axon_ntff_profile_hook = lambda h: _h.__setitem__("hook", h)
    mod.get_axon_ntff_profile_hook = lambda: _h["hook"]
    sys.modules["antenv.axon_hooks"] = mod
    antenv.axon_hooks = mod
    try:
        from trn_agent_boot.trn_boot import _ntff_profile_via_ctypes
        hook = _ntff_profile_via_ctypes("/opt/axon/libaxon_pjrt.so")
        if hook is not None:
            mod.set_axon_ntff_profile_hook(hook)
    except Exception as e:  # profiling degrades, run still works
        print("ntff hook setup failed:", e)


def run_sharded(inputs, trace=False, trace_kwargs=None):
    """Compile (cached) + run on 8 cores. Returns (out [B,N,C] f32, results)."""
    from concourse.bass_utils import run_bass_kernel_spmd
    if trace:
        _ensure_ntff_hook()

    per_core, flags = prepare_core_inputs(**inputs)
    if flags not in _PROG_CACHE:
        _PROG_CACHE[flags] = _build(flags)
    nc = _PROG_CACHE[flags]

    kw = {}
    if trace:
        kw["trace"] = True
        kw["trace_cores"] = [0]
        if trace_kwargs:
            kw["trace_kwargs"] = trace_kwargs
    res = run_bass_kernel_spmd(nc, per_core, core_ids=list(range(B)), **kw)

    out = np.empty((B, N, C), np.float32)
    for b in range(B):
        oT = res.results[b]["outT"]          # [128, KC, N]
        out[b] = oT.transpose(1, 0, 2).reshape(C, N).T
    return out, res


def kernel(**inputs):
    out, _ = run_sharded(inputs, trace=False)
    return out


# revision 22
# speedup vs baseline: 1.0845x; 1.0845x over previous
"""Trainium2 Bass kernel for nn_Block_89361089561275 (dense transformer block).

Sharding: data-parallel over batch B=8 -> one batch element per NeuronCore.
No collectives. Feature-transposed layout (features on SBUF partitions,
tokens on the free dim) so every matmul is a natural lhsT/rhs pair.

v2: fp8(e4m3) weights/activations with DoubleRow perf-mode matmuls (two
contraction chunks per instruction) for QKV/V/proj/MLP/adapter/AV; scores
stay bf16 (K=64 gets no DoubleRow benefit). rpb is passed log-domain in fp8
and folded into softmax via a single DVE (score/64 + rpb) pass, then one
scalar-engine exp -> fp8 P that feeds the AV DoubleRow matmul directly.
LN rstd = exp(-0.5*ln(var+eps)) so the only activation-table sets used are
natural_log_exp (LN1/attention/LN2) and gelu (MLP): 2 table loads total.
Weights are host-prescaled by 8 (fp8 subnormal avoidance); compensations are
folded into the gamma vectors, softmax scale, and activation scale operands.
All matmul accumulation is fp32 in PSUM; residual stream stays fp32.
"""

import sys

for _p in ("/opt/trn_rl_repo",):
    if _p not in sys.path:
        sys.path.insert(0, _p)

import numpy as np
import ml_dtypes

BF16 = ml_dtypes.bfloat16
F8 = ml_dtypes.float8_e4m3

B, N, C, H = 8, 1024, 768, 12
D = C // H            # 64
MLP = 4 * C           # 3072
RED = C // 3          # 256
EPS = 1e-5
P = 128
KC = C // P           # 6   c-chunks
KM = MLP // P         # 24  mlp-chunks
KR = RED // P         # 2   adapter chunks
NT = N // P           # 8   token tiles
HALF = 512
NSL = (slice(0, HALF), slice(HALF, N))
NP8 = N // P          # 8 scattered-stat free width

WS = 8.0              # host weight prescale (fp8 subnormal avoidance)

_PROG_CACHE: dict = {}

# indices into the packed [n, 128, KC] per-feature vector table
V_G1, V_G2, V_QB, V_PB, V_FB, V_L1G, V_L1B, V_L2G, V_L2B = range(9)
NVEC = 9


def _build(flags):
    """Build the single-core Bass program. flags is a tuple of bools:
    (has_mask, qb_nz, vb_nz, pb_nz, f1b_nz, fb_nz, adb_nz,
     ln1_gb_triv, ln2_gb_triv)
    """
    (has_mask, qb_nz, vb_nz, pb_nz, f1b_nz, fb_nz, adb_nz,
     ln1_triv, ln2_triv) = flags

    import concourse.tile as tile
    from concourse import bacc, mybir
    from contextlib import ExitStack

    f32 = mybir.dt.float32
    bf16 = mybir.dt.bfloat16
    f8 = mybir.dt.float8e4
    AF = mybir.ActivationFunctionType
    OP = mybir.AluOpType
    PM = mybir.MatmulPerfMode

    nc = bacc.Bacc("TRN2")

    # ---- external I/O ----
    x_d = nc.declare_dram_parameter("xT", [P, KC, N], f32, isOutput=False)
    rpb_d = nc.declare_dram_parameter("rpbT", [H, N, N], bf16, isOutput=False)
    wqk_d = nc.declare_dram_parameter("wqk", [12, P, KC, P], f8, isOutput=False)
    wv_d = nc.declare_dram_parameter("wv", [P, KC, C], f8, isOutput=False)
    pw_d = nc.declare_dram_parameter("projw", [KC, P, KC, P], f8, isOutput=False)
    f1_d = nc.declare_dram_parameter("fc1w", [KM, P, KC, P], f8, isOutput=False)
    f2_d = nc.declare_dram_parameter("fc2w", [KC, P, KM, P], f8, isOutput=False)
    ad_d = nc.declare_dram_parameter("adw", [KR, P, KC, P], f8, isOutput=False)
    au_d = nc.declare_dram_parameter("auw", [KC, P, KR, P], f8, isOutput=False)
    vec_d = nc.declare_dram_parameter("vecs", [NVEC, P, KC], f32, isOutput=False)
    f1b_d = nc.declare_dram_parameter("fc1b", [P, KM], f32, isOutput=False)
    adb_d = nc.declare_dram_parameter("adb", [P, KR], f32, isOutput=False)
    vb_d = nc.declare_dram_parameter("vbias", [1, C], f32, isOutput=False)
    mb_d = nc.declare_dram_parameter("maskb", [P, NT], f32, isOutput=False)
    out_d = nc.declare_dram_parameter("outT", [P, KC, N], f32, isOutput=True)

    with tile.TileContext(nc) as tc, ExitStack() as ctx:
        sb = ctx.enter_context(tc.tile_pool(name="sb", bufs=1))
        pp = ctx.enter_context(tc.tile_pool(name="pp", bufs=1, space="PSUM"))
        dram = ctx.enter_context(tc.tile_pool(name="dram", bufs=2, space="DRAM"))

        def broadcast(dst, src):
            """dst [p, n] sbuf <- src [1, n] sbuf/psum replicated across
            partitions (via a DRAM bounce)."""
            scratch = dram.tile([1, src.shape[-1]], src.dtype,
                                tag="bscratch", bufs=2, name="bscratch")
            nc.sync.dma_start(out=scratch, in_=src)
            nc.sync.dma_start(out=dst, in_=scratch.to_broadcast(dst.shape))

        def bcast_from_p128(dst, src128):
            """dst [p, n] sbuf <- broadcast of a [128, n/128] scattered row."""
            n = dst.shape[-1]
            scratch = dram.tile([1, n], src128.dtype,
                                tag="bscratch", bufs=2, name="bscratch")
            nc.sync.dma_start(out=scratch, in_=src128)
            nc.sync.dma_start(out=dst, in_=scratch.to_broadcast(dst.shape))

        # ---- persistent tiles ----
        xres = sb.tile([P, KC, N], f32, tag="xres", bufs=1)
        qkT = sb.tile([P, 12, N], bf16, tag="qkT", bufs=1)
        vaug = sb.tile([P, NT // 2, H, 2, D + 16], f8, tag="vaug", bufs=1)
        ones_bf = sb.tile([P, 1], bf16, tag="ones", bufs=1)

        for ch in range(KC):  # per-chunk loads so LN1 stats start early
            nc.sync.dma_start(out=xres[:, ch], in_=x_d[:, ch])
        nc.vector.memset(ones_bf, 1.0)
        nc.vector.memset(vaug, 1.0)

        zero_col = sb.tile([P, 1], f32, tag="zcol", bufs=1)
        nc.vector.memset(zero_col, 0.0)
        eps_col = sb.tile([P, 1], f32, tag="ecol", bufs=1)
        nc.vector.memset(eps_col, float(EPS))

        vecs = sb.tile([P, NVEC, KC], f32, tag="vecs", bufs=1)
        nc.sync.dma_start(out=vecs, in_=vec_d[:].rearrange("v p k -> p v k"))

        def vec(i):
            return vecs[:, i]  # [128, KC]

        if f1b_nz:
            f1b = sb.tile([P, KM], f32, tag="f1b", bufs=1)
            nc.sync.dma_start(out=f1b, in_=f1b_d[:])
        if adb_nz:
            adb = sb.tile([P, KR], f32, tag="adb", bufs=1)
            nc.sync.dma_start(out=adb, in_=adb_d[:])
        if vb_nz:
            vb1 = sb.tile([1, C], f32, tag="vb1", bufs=1)
            nc.sync.dma_start(out=vb1, in_=vb_d[:])
            vb_b = sb.tile([P, C], f32, tag="vb_b", bufs=1)
            broadcast(vb_b, vb1)
        if has_mask:
            maskb = sb.tile([P, NT], f32, tag="maskb", bufs=1)
            nc.sync.dma_start(out=maskb, in_=mb_d[:])

        # ---------------- layernorm (feature-transposed) ----------------
        # Stats via matmul: sum(x) on PSUM row 0, sum(x^2) on row 32.
        def layernorm(dst, g_i, b_i, triv):
            stt = pp.tile([33, N], f32, tag="st", bufs=2, name="ln_stats")
            xbs = []
            for ch in range(KC):
                xb = sb.tile([P, N], bf16, tag="xb", bufs=KC)
                xbs.append(xb)
                nc.vector.tensor_copy(out=xb, in_=xres[:, ch])
                x2 = sb.tile([P, N], bf16, tag="x2", bufs=2)
                nc.scalar.square(out=x2, in_=xres[:, ch])
                for nk in range(2):
                    nc.tensor.matmul(stt[0:1, NSL[nk]], lhsT=ones_bf,
                                     rhs=xb[:, NSL[nk]],
                                     start=(ch == 0), stop=(ch == KC - 1))
                    nc.tensor.matmul(stt[32:33, NSL[nk]], lhsT=ones_bf,
                                     rhs=x2[:, NSL[nk]],
                                     start=(ch == 0), stop=(ch == KC - 1))
            # PSUM rows -> SBUF (DMA cannot read PSUM), then scatter the
            # per-token stats across 128 partitions and finish there
            stat2 = sb.tile([33, N], f32, tag="stat", bufs=2)
            nc.vector.tensor_copy(out=stat2, in_=stt)
            m128 = sb.tile([P, NP8], f32, tag="cs128", bufs=4)
            q128 = sb.tile([P, NP8], f32, tag="cs128", bufs=4)
            nc.sync.dma_start(out=m128, in_=stat2[0:1, :])
            nc.sync.dma_start(out=q128, in_=stat2[32:33, :])
            t128 = sb.tile([P, NP8], f32, tag="cs128", bufs=4)
            nc.vector.tensor_scalar_mul(m128, m128, 1.0 / C)
            nc.vector.tensor_scalar_mul(q128, q128, 1.0 / C)
            nc.vector.tensor_mul(t128, m128, m128)
            nc.vector.tensor_sub(q128, q128, t128)       # var
            # rstd = exp(-0.5*ln(var+eps)): stays in the natural_log_exp
            # activation-table set (no sqrt-set load)
            rn = sb.tile([P, 2, NP8], bf16, tag="cs_bf", bufs=2)
            nc.scalar.activation(t128, q128, AF.Ln, bias=eps_col)
            nc.scalar.activation(rn[:, 1], t128, AF.Exp, scale=-0.5)  # rstd
            # slot 0 <- -mean*rstd
            nc.vector.scalar_tensor_tensor(out=rn[:, 0], in0=m128, scalar=-1.0,
                                           in1=rn[:, 1], op0=OP.mult,
                                           op1=OP.mult)
            # ONE gather+broadcast for both vectors; token n's pair lives at
            # column (n//8)*16 + a*8 + n%8, read back via a strided view
            acb = sb.tile([P, P, 2, NP8], bf16, tag="bcast", bufs=2)
            bcast_from_p128(acb[:].rearrange("p g a k -> p (g a k)"),
                            rn[:].rearrange("p a k -> p (a k)"))
            c_b = acb[:, :, 0, :]
            a_b = acb[:, :, 1, :]
            for ch in range(KC):
                t1 = sb.tile([P, P, NP8], bf16, tag="x2", bufs=2)
                xb3 = xbs[ch][:].rearrange("p (g k) -> p g k", k=NP8)
                nc.vector.tensor_mul(t1, xb3, a_b)
                dst3 = dst[:, ch].rearrange("p (g k) -> p g k", k=NP8)
                if triv:
                    nc.vector.tensor_add(dst3, t1, c_b)
                else:
                    nc.vector.tensor_add(t1, t1, c_b)
                    nc.vector.tensor_scalar(
                        out=dst[:, ch],
                        in0=t1[:].rearrange("p g k -> p (g k)"),
                        scalar1=vec(g_i)[:, ch:ch + 1],
                        scalar2=vec(b_i)[:, ch:ch + 1],
                        op0=OP.mult, op1=OP.add)

        # ---------------- LN1 + QKV ----------------
        xnT = sb.tile([P, KC, N], f8, tag="feat", bufs=2)
        layernorm(xnT, V_L1G, V_L1B, ln1_triv)
        # pair-contiguous per-token-tile copy so the v matmul can DoubleRow
        xnv = sb.tile([P, NT, KC // 2, 2, P], f8, tag="xnv", bufs=1)
        for ch in range(KC):
            nc.vector.tensor_copy(
                out=xnv[:, :, ch // 2, ch % 2, :],
                in_=xnT[:, ch].rearrange("p (t w) -> p t w", w=P))

        wv_sb = sb.tile([P, KC, C], f8, tag="wv", bufs=1)
        nc.sync.dma_start(out=wv_sb, in_=wv_d[:])

        def qk_block(blk):
            wt = sb.tile([P, KC, P], f8, tag="w6", bufs=3)
            nc.sync.dma_start(out=wt, in_=wqk_d[blk])
            for nk in range(2):
                mm = pp.tile([P, HALF], f32, tag="st", bufs=2)
                for kp in range(KC // 2):
                    nc.tensor.matmul(mm, lhsT=wt[:, 2 * kp:2 * kp + 2],
                                     rhs=xnT[:, 2 * kp:2 * kp + 2, NSL[nk]],
                                     start=(kp == 0), stop=(kp == KC // 2 - 1),
                                     perf_mode=PM.DoubleRow)
                dst = qkT[:, blk, NSL[nk]]
                if blk < 6 and qb_nz:
                    nc.vector.tensor_scalar_add(dst, mm, vec(V_QB)[:, blk:blk + 1])
                else:
                    nc.scalar.copy(out=dst, in_=mm)

        def v_tile(t):
            for off, cw in ((0, HALF), (HALF, C - HALF)):
                mm = pp.tile([P, HALF], f32, tag="st", bufs=2)
                for kp in range(KC // 2):
                    nc.tensor.matmul(
                        mm[:, :cw],
                        lhsT=xnv[:, t, kp],
                        rhs=wv_sb[:, 2 * kp:2 * kp + 2, off:off + cw],
                        start=(kp == 0), stop=(kp == KC // 2 - 1),
                        perf_mode=PM.DoubleRow)
                dst = vaug[:, t // 2, off // D:(off + cw) // D, t % 2, :D]
                src = mm[:, :cw].rearrange("p (h d) -> p h d", d=D)
                if vb_nz:
                    nc.vector.tensor_add(
                        dst, src,
                        vb_b[:, off:off + cw].rearrange("p (h d) -> p h d", d=D))
                else:
                    nc.scalar.copy(out=dst, in_=src)

        # head-pair 0's q/k first; v tiles and the remaining q/k blocks are
        # interleaved into the attention loop (they fill the PE while the
        # softmax chain runs, and attention starts ~10us earlier)
        qk_block(0)
        qk_block(6)

        # ---------------- attention ----------------
        # Heads (2j, 2j+1) at partition offsets 0/64 of qkT block j; the two
        # K=64 score matmuls run in distinct PE row-groups. P~ = exp(s/64+rpb)
        # computed as one DVE pass (score scale + log-rpb add) + one scalar
        # exp straight to fp8, feeding the AV DoubleRow matmul (2 m-tiles per
        # instruction).
        oT = sb.tile([P, KC, N], f8, tag="feat", bufs=2)

        def evac_head(o_ps, hp, hh):
            # unnormalized head in PSUM [65, 512] x2; row 64 = colsum.
            # Copy to SBUF right away (frees the accumulator banks; DMA
            # cannot read PSUM; scalar engine has slack here), broadcast
            # the colsum, normalize async.
            ou = sb.tile([D + 16, N], f32, tag="stat", bufs=2)
            for nk in range(2):
                nc.vector.tensor_copy(out=ou[:, NSL[nk]], in_=o_ps[nk])
            raw = sb.tile([P, N], f32, tag="rec", bufs=2)
            broadcast(raw[0:D, :], ou[D:D + 1, :])
            rb = sb.tile([P, N], f32, tag="rec2", bufs=2)
            nc.vector.reciprocal_approx_fast(out=rb[0:D, :], in_=raw[0:D, :])
            ot_tmp = sb.tile([P, N], f8, tag="ott", bufs=2)
            # 16/colsum * o_unnorm (the 16 re-centers fp8; undone in g1)
            nc.vector.scalar_tensor_tensor(
                out=ot_tmp[0:D, :], in0=ou[0:D, :], scalar=16.0,
                in1=rb[0:D, :], op0=OP.mult, op1=OP.mult)
            nc.sync.dma_start(out=oT[hh * D:(hh + 1) * D, hp, :],
                              in_=ot_tmp[0:D, :])

        for hp in range(H // 2):
            qh = [qkT[hh * D:(hh + 1) * D, hp, :] for hh in range(2)]
            kh = [qkT[hh * D:(hh + 1) * D, 6 + hp, :] for hh in range(2)]
            o_ps = [[pp.tile([D + 16, HALF], f32, tag="acc", bufs=4,
                             name=f"o_ps{hh}{nk}") for nk in range(2)]
                    for hh in range(2)]
            p8 = [sb.tile([P, 2, N], f8, tag="p8", bufs=4,
                          name=f"p8_{hh}") for hh in range(2)]
            for mt in range(NT):
                if hp == 0:
                    v_tile(mt)
                if hp < H // 2 - 1:
                    if mt == 4:
                        qk_block(hp + 1)
                    elif mt == 6:
                        qk_block(hp + 7)
                for hh in range(2):
                    h = 2 * hp + hh
                    rpb_t = sb.tile([P, N], bf16, tag="rpb", bufs=6)
                    nc.sync.dma_start(out=rpb_t,
                                      in_=rpb_d[h, mt * P:(mt + 1) * P, :])
                    st = pp.tile([P, N], f32, tag="st", bufs=2, name="st")
                    for nk in range(2):
                        nc.tensor.matmul(st[:, NSL[nk]],
                                         lhsT=kh[hh][:, mt * P:(mt + 1) * P],
                                         rhs=qh[hh][:, NSL[nk]],
                                         start=True, stop=True)
                    # P~ = exp(s/512)*exp(rpb): scalar reads PSUM with its
                    # free affine scale; DVE does an all-16-bit multiply
                    texp = sb.tile([P, N], bf16, tag="texp", bufs=3)
                    nc.scalar.activation(out=texp, in_=st, func=AF.Exp,
                                         scale=1.0 / 512.0)
                    pslot = p8[hh][:, mt % 2, :]
                    if has_mask:
                        nc.vector.scalar_tensor_tensor(
                            out=pslot, in0=texp, scalar=maskb[:, mt:mt + 1],
                            in1=rpb_t, op0=OP.mult, op1=OP.mult)
                    elif mt in (0, 2, 4):
                        # offload 6/16 of the P=exp(s)*exp(rpb) multiplies per
                        # head-pair to the otherwise-idle GpSimd engine; only
                        # even-mt tiles (their AV consumer is a full mul-time
                        # away, so GpSimd's ~2.4us latency stays off-path)
                        nc.gpsimd.tensor_tensor(out=pslot, in0=texp,
                                                in1=rpb_t, op=OP.mult)
                    else:
                        nc.vector.tensor_mul(pslot, texp, rpb_t)
                if mt % 2 == 1:
                    for hh in range(2):
                        for nk in range(2):
                            nc.tensor.matmul(
                                o_ps[hh][nk][:, :],
                                lhsT=vaug[:, mt // 2, 2 * hp + hh, :, :],
                                rhs=p8[hh][:, :, NSL[nk]],
                                start=(mt == 1), stop=(mt == NT - 1),
                                perf_mode=PM.DoubleRow)
                    if mt < NT - 1:
                        p8 = [sb.tile([P, 2, N], f8, tag="p8", bufs=4,
                                      name=f"p8_{hh}") for hh in range(2)]
            evac_head(o_ps[0], hp, 0)
            evac_head(o_ps[1], hp, 1)

        # ---------------- proj + residual 1 ----------------
        for mt in range(KC):
            wt = sb.tile([P, KC, P], f8, tag="w6", bufs=3)
            nc.sync.dma_start(out=wt, in_=pw_d[mt])
            for nk in range(2):
                mm = pp.tile([P, HALF], f32, tag="acc", bufs=4)
                for kp in range(KC // 2):
                    nc.tensor.matmul(mm, lhsT=wt[:, 2 * kp:2 * kp + 2],
                                     rhs=oT[:, 2 * kp:2 * kp + 2, NSL[nk]],
                                     start=(kp == 0), stop=(kp == KC // 2 - 1),
                                     perf_mode=PM.DoubleRow)
                if pb_nz:
                    nc.vector.tensor_scalar_add(mm, mm, vec(V_PB)[:, mt:mt + 1])
                nc.vector.scalar_tensor_tensor(
                    out=xres[:, mt, NSL[nk]], in0=mm,
                    scalar=vec(V_G1)[:, mt:mt + 1],
                    in1=xres[:, mt, NSL[nk]], op0=OP.mult, op1=OP.add)

        # ---------------- LN2, adapter-down, MLP ----------------
        xn2T = sb.tile([P, KC, N], f8, tag="feat", bufs=2)
        layernorm(xn2T, V_L2G, V_L2B, ln2_triv)

        a1T = sb.tile([P, KR, N], f8, tag="a1", bufs=1)
        for mt in range(KR):
            wt = sb.tile([P, KC, P], f8, tag="w6", bufs=3)
            nc.sync.dma_start(out=wt, in_=ad_d[mt])
            for nk in range(2):
                mm = pp.tile([P, HALF], f32, tag="acc", bufs=4)
                for kp in range(KC // 2):
                    nc.tensor.matmul(mm, lhsT=wt[:, 2 * kp:2 * kp + 2],
                                     rhs=xn2T[:, 2 * kp:2 * kp + 2, NSL[nk]],
                                     start=(kp == 0), stop=(kp == KC // 2 - 1),
                                     perf_mode=PM.DoubleRow)
                nc.scalar.activation(
                    out=a1T[:, mt, NSL[nk]], in_=mm, func=AF.Relu,
                    scale=1.0 / WS,
                    bias=(adb[:, mt:mt + 1] if adb_nz else zero_col))

        # fc1 for BOTH token halves interleaved: 12 DoubleRow matmuls per
        # gelu-pair window keeps the PE dense (max p-state); each fc1 weight
        # tile is loaded once. Two output blocks share one 2-bank PSUM tile
        # so a single [128,1024] gelu covers them.
        h1s = [sb.tile([P, KM, HALF], f8, tag=f"h1_{nk}", bufs=1,
                       name=f"h1_{nk}") for nk in range(2)]
        for mtp in range(KM // 2):
            wts = []
            for j in range(2):
                wt = sb.tile([P, KC, P], f8, tag="w6", bufs=3)
                nc.sync.dma_start(out=wt, in_=f1_d[2 * mtp + j])
                wts.append(wt)
            for nk in range(2):
                mm2 = pp.tile([P, N], f32, tag="st", bufs=2)
                for j in range(2):
                    for kp in range(KC // 2):
                        nc.tensor.matmul(mm2[:, j * HALF:(j + 1) * HALF],
                                         lhsT=wts[j][:, 2 * kp:2 * kp + 2],
                                         rhs=xn2T[:, 2 * kp:2 * kp + 2, NSL[nk]],
                                         start=(kp == 0),
                                         stop=(kp == KC // 2 - 1),
                                         perf_mode=PM.DoubleRow)
                h1 = h1s[nk]
                if f1b_nz:
                    for j in range(2):
                        mt = 2 * mtp + j
                        nc.scalar.activation(
                            out=h1[:, mt], in_=mm2[:, j * HALF:(j + 1) * HALF],
                            func=AF.Gelu, scale=1.0 / WS,
                            bias=f1b[:, mt:mt + 1])
                else:
                    nc.scalar.activation(
                        out=h1[:, 2 * mtp:2 * mtp + 2].rearrange(
                            "p a t -> p (a t)"),
                        in_=mm2, func=AF.Gelu, scale=1.0 / WS, bias=zero_col)
        for mt in range(KC):
            w2 = sb.tile([P, KM, P], f8, tag="w24", bufs=2)
            nc.sync.dma_start(out=w2, in_=f2_d[mt])
            au = sb.tile([P, KR, P], f8, tag="w2", bufs=2)
            nc.sync.dma_start(out=au, in_=au_d[mt])
            for nk in range(2):
                mm = pp.tile([P, HALF], f32, tag="acc", bufs=4)
                for kp in range(KM // 2):
                    nc.tensor.matmul(mm, lhsT=w2[:, 2 * kp:2 * kp + 2],
                                     rhs=h1s[nk][:, 2 * kp:2 * kp + 2],
                                     start=(kp == 0), stop=False,
                                     perf_mode=PM.DoubleRow)
                nc.tensor.matmul(mm, lhsT=au, rhs=a1T[:, :, NSL[nk]],
                                 start=False, stop=True,
                                 perf_mode=PM.DoubleRow)
                if fb_nz:
                    nc.vector.tensor_scalar_add(mm, mm, vec(V_FB)[:, mt:mt + 1])
                nc.vector.scalar_tensor_tensor(
                    out=xres[:, mt, NSL[nk]], in0=mm,
                    scalar=vec(V_G2)[:, mt:mt + 1],
                    in1=xres[:, mt, NSL[nk]], op0=OP.mult, op1=OP.add)
                nc.sync.dma_start(out=out_d[:, mt, NSL[nk]],
                                  in_=xres[:, mt, NSL[nk]])

    if not nc.is_finalized():
        nc.finalize()
    return nc


def _pack_w6(wT, km, kk):
    """[K, M] (K=contraction, M=out) -> [M//128, 128, K//128, 128] tiles
    laid out so each DMA partition read is contiguous."""
    K, M = wT.shape
    assert K == kk * P and M == km * P
    a = wT.reshape(kk, P, km, P)          # [ks, p, mt, col]
    return np.ascontiguousarray(a.transpose(2, 1, 0, 3)).astype(F8)


def _stripe(v, k):
    """[k*128] -> [128, k] with v[ks*128+p] at [p, ks]."""
    return np.ascontiguousarray(v.reshape(k, P).T).astype(np.float32)


def prepare_core_inputs(x, mask, rpb, ln1_g, ln1_b, qkv_w, q_bias, v_bias,
                        proj_w, proj_b, gamma1, ln2_g, ln2_b, fc1_w, fc1_b,
                        fc2_w, fc2_b, ad_dw, ad_db, ad_uw, ad_ub, gamma2):
    """Host-side layout prep. Returns (per_core_maps, flags)."""
    f32 = np.float32

    qkv_w = np.asarray(qkv_w, f32)
    wq = qkv_w[:C] * WS            # both x8; with 1/sqrt(D)=1/8 the scores
    wk = qkv_w[C:2 * C] * WS       # need a 1/(8*8*8)=1/512 softmax-side scale
    wv = qkv_w[2 * C:] * WS
    wqkT = np.concatenate([wq, wk], 0).T          # [C, 1536]
    wqk = _pack_w6(wqkT, 12, KC)
    # wv used as matmul rhs: [p, ks, col] = wv[col, ks*128+p]
    wv_packed = np.ascontiguousarray(
        wv.T.reshape(KC, P, C).transpose(1, 0, 2)).astype(F8)

    projw = _pack_w6(np.asarray(proj_w, f32).T * WS, KC, KC)
    fc1w = _pack_w6(np.asarray(fc1_w, f32).T * WS, KM, KC)
    fc2w = _pack_w6(np.asarray(fc2_w, f32).T * WS, KC, KM)
    adw = _pack_w6(np.asarray(ad_dw, f32).T * WS, KR, KC)
    auw = _pack_w6(np.asarray(ad_uw, f32).T * WS, KC, KR)

    # exp(rpb) so the kernel folds the bias into softmax as a multiply
    rpbT = np.ascontiguousarray(
        np.exp(np.asarray(rpb, f32).transpose(0, 2, 1))).astype(BF16)

    fb = (np.asarray(fc2_b, f32) + np.asarray(ad_ub, f32)) * WS

    # gamma1 compensates: v x8, evac x16, projw x8  -> /1024
    # gamma2 compensates: fc2w/auw x8 -> /8; fb prescaled x8 to match
    vecs = np.stack([
        _stripe(np.asarray(gamma1, f32) / (WS * 16.0 * WS), KC),
        _stripe(np.asarray(gamma2, f32) / WS, KC),
        _stripe(np.asarray(q_bias, f32) * WS, KC),
        _stripe(np.asarray(proj_b, f32) * (WS * 16.0 * WS), KC),
        _stripe(fb, KC),
        _stripe(np.asarray(ln1_g, f32), KC),
        _stripe(np.asarray(ln1_b, f32), KC),
        _stripe(np.asarray(ln2_g, f32), KC),
        _stripe(np.asarray(ln2_b, f32), KC),
    ], 0)  # [NVEC, 128, KC]

    f1b = _stripe(np.asarray(fc1_b, f32), KM)
    adb = _stripe(np.asarray(ad_db, f32), KR)
    vb = (np.asarray(v_bias, f32) * WS).reshape(1, C).astype(f32)

    mask = np.asarray(mask)
    has_mask = not bool(mask.all())

    flags = (
        has_mask,
        bool(np.any(np.asarray(q_bias, f32))),
        bool(np.any(v_bias)),
        bool(np.any(proj_b)),
        bool(np.any(fc1_b)),
        bool(np.any(fb)),
        bool(np.any(ad_db)),
        bool(np.all(ln1_g == 1.0) and not np.any(ln1_b)),
        bool(np.all(ln2_g == 1.0) and not np.any(ln2_b)),
    )

    shared = {
        "rpbT": rpbT, "wqk": wqk, "wv": wv_packed, "projw": projw,
        "fc1w": fc1w, "fc2w": fc2w, "adw": adw, "auw": auw,
        "vecs": vecs, "fc1b": f1b, "adb": adb, "vbias": vb,
    }

    x = np.asarray(x, f32)
    per_core = []
    for b in range(B):
        xT = np.ascontiguousarray(
            x[b].T.reshape(KC, P, N).transpose(1, 0, 2)).astype(f32)
        if has_mask:
            mb = np.where(mask[b], 1.0, 0.0).astype(f32)    # [N] over keys m
            mb = np.ascontiguousarray(mb.reshape(NT, P).T)  # [128, NT]
        else:
            mb = np.zeros((P, NT), f32)
        m = dict(shared)
        m["xT"] = xT
        m["maskb"] = mb
        per_core.append(m)
    return per_core, flags


def _ensure_ntff_hook():
    """The agent image lacks ``antenv.axon_hooks``; provide it and register
    the ctypes NTFF profile hook so trace=True works under axon."""
    import types
    try:
        from antenv.axon_hooks import get_axon_ntff_profile_hook  # noqa: F401
        return
    except ImportError:
        pass
    import antenv
    mod = types.ModuleType("antenv.axon_hooks")
    _h = {"hook": None}
    mod.set_axon_ntff_profile_hook = lambda h: _h.__setitem__("hook", h)
    mod.get_axon_ntff_profile_hook = lambda: _h["hook"]
    sys.modules["antenv.axon_hooks"] = mod
    antenv.axon_hooks = mod
    try:
        from trn_agent_boot.trn_boot import _ntff_profile_via_ctypes
        hook = _ntff_profile_via_ctypes("/opt/axon/libaxon_pjrt.so")
        if hook is not None:
            mod.set_axon_ntff_profile_hook(hook)
    except Exception as e:  # profiling degrades, run still works
        print("ntff hook setup failed:", e)


def run_sharded(inputs, trace=False, trace_kwargs=None):
    """Compile (cached) + run on 8 cores. Returns (out [B,N,C] f32, results)."""
    from concourse.bass_utils import run_bass_kernel_spmd
    if trace:
        _ensure_ntff_hook()

    per_core, flags = prepare_core_inputs(**inputs)
    if flags not in _PROG_CACHE:
        _PROG_CACHE[flags] = _build(flags)
    nc = _PROG_CACHE[flags]

    kw = {}
    if trace:
        kw["trace"] = True
        kw["trace_cores"] = [0]
        if trace_kwargs:
            kw["trace_kwargs"] = trace_kwargs
    res = run_bass_kernel_spmd(nc, per_core, core_ids=list(range(B)), **kw)

    out = np.empty((B, N, C), np.float32)
    for b in range(B):
        oT = res.results[b]["outT"]          # [128, KC, N]
        out[b] = oT.transpose(1, 0, 2).reshape(C, N).T
    return out, res


def kernel(**inputs):
    out, _ = run_sharded(inputs, trace=False)
    return out


# revision 23
# speedup vs baseline: 1.1207x; 1.0334x over previous
"""Trainium2 Bass kernel for nn_Block_89361089561275 (dense transformer block).

Sharding: data-parallel over batch B=8 -> one batch element per NeuronCore.
No collectives. Feature-transposed layout (features on SBUF partitions,
tokens on the free dim) so every matmul is a natural lhsT/rhs pair.

v2: fp8(e4m3) weights/activations with DoubleRow perf-mode matmuls (two
contraction chunks per instruction) for QKV/V/proj/MLP/adapter/AV; scores
stay bf16 (K=64 gets no DoubleRow benefit). rpb is passed log-domain in fp8
and folded into softmax via a single DVE (score/64 + rpb) pass, then one
scalar-engine exp -> fp8 P that feeds the AV DoubleRow matmul directly.
LN rstd = exp(-0.5*ln(var+eps)) so the only activation-table sets used are
natural_log_exp (LN1/attention/LN2) and gelu (MLP): 2 table loads total.
Weights are host-prescaled by 8 (fp8 subnormal avoidance); compensations are
folded into the gamma vectors, softmax scale, and activation scale operands.
All matmul accumulation is fp32 in PSUM; residual stream stays fp32.
"""

import sys

for _p in ("/opt/trn_rl_repo",):
    if _p not in sys.path:
        sys.path.insert(0, _p)

import numpy as np
import ml_dtypes

BF16 = ml_dtypes.bfloat16
F8 = ml_dtypes.float8_e4m3

B, N, C, H = 8, 1024, 768, 12
D = C // H            # 64
MLP = 4 * C           # 3072
RED = C // 3          # 256
EPS = 1e-5
P = 128
KC = C // P           # 6   c-chunks
KM = MLP // P         # 24  mlp-chunks
KR = RED // P         # 2   adapter chunks
NT = N // P           # 8   token tiles
HALF = 512
NSL = (slice(0, HALF), slice(HALF, N))
NP8 = N // P          # 8 scattered-stat free width

WS = 8.0              # host weight prescale (fp8 subnormal avoidance)

_PROG_CACHE: dict = {}

# indices into the packed [n, 128, KC] per-feature vector table
V_G1, V_G2, V_QB, V_PB, V_FB, V_L1G, V_L1B, V_L2G, V_L2B = range(9)
NVEC = 9


def _build(flags):
    """Build the single-core Bass program. flags is a tuple of bools:
    (has_mask, qb_nz, vb_nz, pb_nz, f1b_nz, fb_nz, adb_nz,
     ln1_gb_triv, ln2_gb_triv)
    """
    (has_mask, qb_nz, vb_nz, pb_nz, f1b_nz, fb_nz, adb_nz,
     ln1_triv, ln2_triv) = flags

    import concourse.tile as tile
    from concourse import bacc, mybir
    from contextlib import ExitStack

    f32 = mybir.dt.float32
    bf16 = mybir.dt.bfloat16
    f8 = mybir.dt.float8e4
    AF = mybir.ActivationFunctionType
    OP = mybir.AluOpType
    PM = mybir.MatmulPerfMode

    nc = bacc.Bacc("TRN2")

    # ---- external I/O ----
    x_d = nc.declare_dram_parameter("xT", [P, KC, N], f32, isOutput=False)
    rpb_d = nc.declare_dram_parameter("rpbT", [H, N, N], bf16, isOutput=False)
    wqk_d = nc.declare_dram_parameter("wqk", [12, P, KC, P], f8, isOutput=False)
    wv_d = nc.declare_dram_parameter("wv", [P, KC, C], f8, isOutput=False)
    pw_d = nc.declare_dram_parameter("projw", [KC, P, KC, P], f8, isOutput=False)
    f1_d = nc.declare_dram_parameter("fc1w", [KM, P, KC, P], f8, isOutput=False)
    f2_d = nc.declare_dram_parameter("fc2w", [KC, P, KM, P], f8, isOutput=False)
    ad_d = nc.declare_dram_parameter("adw", [KR, P, KC, P], f8, isOutput=False)
    au_d = nc.declare_dram_parameter("auw", [KC, P, KR, P], f8, isOutput=False)
    vec_d = nc.declare_dram_parameter("vecs", [NVEC, P, KC], f32, isOutput=False)
    f1b_d = nc.declare_dram_parameter("fc1b", [P, KM], f32, isOutput=False)
    adb_d = nc.declare_dram_parameter("adb", [P, KR], f32, isOutput=False)
    vb_d = nc.declare_dram_parameter("vbias", [1, C], f32, isOutput=False)
    mb_d = nc.declare_dram_parameter("maskb", [P, NT], f32, isOutput=False)
    out_d = nc.declare_dram_parameter("outT", [P, KC, N], f32, isOutput=True)

    with tile.TileContext(nc) as tc, ExitStack() as ctx:
        sb = ctx.enter_context(tc.tile_pool(name="sb", bufs=1))
        pp = ctx.enter_context(tc.tile_pool(name="pp", bufs=1, space="PSUM"))
        dram = ctx.enter_context(tc.tile_pool(name="dram", bufs=2, space="DRAM"))

        def broadcast(dst, src):
            """dst [p, n] sbuf <- src [1, n] sbuf/psum replicated across
            partitions (via a DRAM bounce)."""
            scratch = dram.tile([1, src.shape[-1]], src.dtype,
                                tag="bscratch", bufs=2, name="bscratch")
            nc.sync.dma_start(out=scratch, in_=src)
            nc.sync.dma_start(out=dst, in_=scratch.to_broadcast(dst.shape))

        def bcast_from_p128(dst, src128):
            """dst [p, n] sbuf <- broadcast of a [128, n/128] scattered row."""
            n = dst.shape[-1]
            scratch = dram.tile([1, n], src128.dtype,
                                tag="bscratch", bufs=2, name="bscratch")
            nc.sync.dma_start(out=scratch, in_=src128)
            nc.sync.dma_start(out=dst, in_=scratch.to_broadcast(dst.shape))

        # ---- persistent tiles ----
        xres = sb.tile([P, KC, N], f32, tag="xres", bufs=1)
        qkT = sb.tile([P, 12, N], bf16, tag="qkT", bufs=1)
        vaug = sb.tile([P, NT // 2, H, 2, D + 16], f8, tag="vaug", bufs=1)
        ones_bf = sb.tile([P, 1], bf16, tag="ones", bufs=1)

        for ch in range(KC):  # per-chunk loads so LN1 stats start early
            nc.sync.dma_start(out=xres[:, ch], in_=x_d[:, ch])
        nc.vector.memset(ones_bf, 1.0)
        nc.vector.memset(vaug, 1.0)

        zero_col = sb.tile([P, 1], f32, tag="zcol", bufs=1)
        nc.vector.memset(zero_col, 0.0)
        i32 = mybir.dt.int32
        magic = sb.tile([P, NP8], i32, tag="magic", bufs=1)
        nc.vector.memset(magic, 0x5f3759df)

        vecs = sb.tile([P, NVEC, KC], f32, tag="vecs", bufs=1)
        nc.sync.dma_start(out=vecs, in_=vec_d[:].rearrange("v p k -> p v k"))

        def vec(i):
            return vecs[:, i]  # [128, KC]

        if f1b_nz:
            f1b = sb.tile([P, KM], f32, tag="f1b", bufs=1)
            nc.sync.dma_start(out=f1b, in_=f1b_d[:])
        if adb_nz:
            adb = sb.tile([P, KR], f32, tag="adb", bufs=1)
            nc.sync.dma_start(out=adb, in_=adb_d[:])
        if vb_nz:
            vb1 = sb.tile([1, C], f32, tag="vb1", bufs=1)
            nc.sync.dma_start(out=vb1, in_=vb_d[:])
            vb_b = sb.tile([P, C], f32, tag="vb_b", bufs=1)
            broadcast(vb_b, vb1)
        if has_mask:
            maskb = sb.tile([P, NT], f32, tag="maskb", bufs=1)
            nc.sync.dma_start(out=maskb, in_=mb_d[:])

        # ---------------- layernorm (feature-transposed) ----------------
        # Stats via matmul: sum(x) on PSUM row 0, sum(x^2) on row 32.
        def layernorm(dst, g_i, b_i, triv):
            stt = pp.tile([33, N], f32, tag="st", bufs=2, name="ln_stats")
            xbs = []
            for ch in range(KC):
                xb = sb.tile([P, N], bf16, tag="xb", bufs=KC)
                xbs.append(xb)
                nc.vector.tensor_copy(out=xb, in_=xres[:, ch])
                x2 = sb.tile([P, N], bf16, tag="x2", bufs=2)
                nc.scalar.square(out=x2, in_=xres[:, ch])
                for nk in range(2):
                    nc.tensor.matmul(stt[0:1, NSL[nk]], lhsT=ones_bf,
                                     rhs=xb[:, NSL[nk]],
                                     start=(ch == 0), stop=(ch == KC - 1))
                    nc.tensor.matmul(stt[32:33, NSL[nk]], lhsT=ones_bf,
                                     rhs=x2[:, NSL[nk]],
                                     start=(ch == 0), stop=(ch == KC - 1))
            # PSUM rows -> SBUF (DMA cannot read PSUM), then scatter the
            # per-token stats across 128 partitions and finish there
            stat2 = sb.tile([33, N], f32, tag="stat", bufs=2)
            nc.vector.tensor_copy(out=stat2, in_=stt)
            m128 = sb.tile([P, NP8], f32, tag="cs128", bufs=4)
            q128 = sb.tile([P, NP8], f32, tag="cs128", bufs=4)
            nc.sync.dma_start(out=m128, in_=stat2[0:1, :])
            nc.sync.dma_start(out=q128, in_=stat2[32:33, :])
            t128 = sb.tile([P, NP8], f32, tag="cs128", bufs=4)
            r128 = sb.tile([P, NP8], f32, tag="cs128", bufs=4)
            nc.vector.tensor_scalar_mul(m128, m128, 1.0 / C)
            nc.vector.tensor_scalar(out=q128, in0=q128, scalar1=1.0 / C,
                                    scalar2=float(EPS), op0=OP.mult,
                                    op1=OP.add)
            nc.vector.tensor_mul(t128, m128, m128)
            nc.vector.tensor_sub(q128, q128, t128)       # var + eps
            # rstd = 1/sqrt(var+eps) fully on DVE (bit-twiddle seed + one
            # Newton step) -- no scalar activation, no table loads
            hsh = sb.tile([P, NP8], i32, tag="cs_i", bufs=2)
            nc.vector.tensor_scalar(out=hsh, in0=q128.bitcast(i32), scalar1=1,
                                    scalar2=None, op0=OP.logical_shift_right)
            y0i = sb.tile([P, NP8], i32, tag="cs_i", bufs=2)
            nc.vector.tensor_tensor(out=y0i, in0=magic, in1=hsh,
                                    op=OP.subtract)
            y0 = y0i.bitcast(f32)
            nc.vector.tensor_mul(t128, y0, y0)
            nc.vector.tensor_mul(t128, t128, q128)
            nc.vector.tensor_scalar(out=t128, in0=t128, scalar1=-0.5,
                                    scalar2=1.5, op0=OP.mult, op1=OP.add)
            nc.vector.tensor_mul(r128, y0, t128)         # rstd
            r_bf = sb.tile([P, NP8], bf16, tag="cs_bf", bufs=2)
            n_bf = sb.tile([P, NP8], bf16, tag="cs_bf", bufs=2)
            nc.vector.tensor_copy(out=r_bf, in_=r128)
            # -mean*rstd straight to bf16
            nc.vector.scalar_tensor_tensor(out=n_bf, in0=m128, scalar=-1.0,
                                           in1=r128, op0=OP.mult, op1=OP.mult)
            a_b = sb.tile([P, N], bf16, tag="bcast", bufs=2)
            c_b = sb.tile([P, N], bf16, tag="bcast", bufs=2)
            bcast_from_p128(a_b, r_bf)
            bcast_from_p128(c_b, n_bf)
            for ch in range(KC):
                t1 = sb.tile([P, N], bf16, tag="x2", bufs=2)
                nc.vector.tensor_mul(t1, xbs[ch], a_b)
                if triv:
                    nc.vector.tensor_add(dst[:, ch], t1, c_b)
                else:
                    nc.vector.tensor_add(t1, t1, c_b)
                    nc.vector.tensor_scalar(
                        out=dst[:, ch], in0=t1,
                        scalar1=vec(g_i)[:, ch:ch + 1],
                        scalar2=vec(b_i)[:, ch:ch + 1],
                        op0=OP.mult, op1=OP.add)

        # ---------------- LN1 + QKV ----------------
        xnT = sb.tile([P, KC, N], f8, tag="feat", bufs=2)
        layernorm(xnT, V_L1G, V_L1B, ln1_triv)
        # pair-contiguous per-token-tile copy so the v matmul can DoubleRow
        xnv = sb.tile([P, NT, KC // 2, 2, P], f8, tag="xnv", bufs=1)
        for ch in range(KC):
            nc.vector.tensor_copy(
                out=xnv[:, :, ch // 2, ch % 2, :],
                in_=xnT[:, ch].rearrange("p (t w) -> p t w", w=P))

        wv_sb = sb.tile([P, KC, C], f8, tag="wv", bufs=1)
        nc.sync.dma_start(out=wv_sb, in_=wv_d[:])

        def qk_block(blk):
            wt = sb.tile([P, KC, P], f8, tag="w6", bufs=3)
            nc.sync.dma_start(out=wt, in_=wqk_d[blk])
            for nk in range(2):
                mm = pp.tile([P, HALF], f32, tag="st", bufs=2)
                for kp in range(KC // 2):
                    nc.tensor.matmul(mm, lhsT=wt[:, 2 * kp:2 * kp + 2],
                                     rhs=xnT[:, 2 * kp:2 * kp + 2, NSL[nk]],
                                     start=(kp == 0), stop=(kp == KC // 2 - 1),
                                     perf_mode=PM.DoubleRow)
                dst = qkT[:, blk, NSL[nk]]
                if blk < 6 and qb_nz:
                    nc.vector.tensor_scalar_add(dst, mm, vec(V_QB)[:, blk:blk + 1])
                else:
                    nc.scalar.copy(out=dst, in_=mm)

        def v_tile(t):
            for off, cw in ((0, HALF), (HALF, C - HALF)):
                mm = pp.tile([P, HALF], f32, tag="st", bufs=2)
                for kp in range(KC // 2):
                    nc.tensor.matmul(
                        mm[:, :cw],
                        lhsT=xnv[:, t, kp],
                        rhs=wv_sb[:, 2 * kp:2 * kp + 2, off:off + cw],
                        start=(kp == 0), stop=(kp == KC // 2 - 1),
                        perf_mode=PM.DoubleRow)
                dst = vaug[:, t // 2, off // D:(off + cw) // D, t % 2, :D]
                src = mm[:, :cw].rearrange("p (h d) -> p h d", d=D)
                if vb_nz:
                    nc.vector.tensor_add(
                        dst, src,
                        vb_b[:, off:off + cw].rearrange("p (h d) -> p h d", d=D))
                else:
                    nc.scalar.copy(out=dst, in_=src)

        # head-pair 0's q/k first; v tiles and the remaining q/k blocks are
        # interleaved into the attention loop (they fill the PE while the
        # softmax chain runs, and attention starts ~10us earlier)
        qk_block(0)
        qk_block(6)

        # ---------------- attention ----------------
        # Heads (2j, 2j+1) at partition offsets 0/64 of qkT block j; the two
        # K=64 score matmuls run in distinct PE row-groups. P~ = exp(s/64+rpb)
        # computed as one DVE pass (score scale + log-rpb add) + one scalar
        # exp straight to fp8, feeding the AV DoubleRow matmul (2 m-tiles per
        # instruction).
        oT = sb.tile([P, KC, N], f8, tag="feat", bufs=2)

        def evac_head(o_ps, hp, hh):
            # unnormalized head in PSUM [65, 512] x2; row 64 = colsum.
            # Copy to SBUF right away (frees the accumulator banks; DMA
            # cannot read PSUM; scalar engine has slack here), broadcast
            # the colsum, normalize async.
            ou = sb.tile([D + 16, N], f32, tag="stat", bufs=2)
            for nk in range(2):
                nc.vector.tensor_copy(out=ou[:, NSL[nk]], in_=o_ps[nk])
            raw = sb.tile([P, N], f32, tag="rec", bufs=2)
            broadcast(raw[0:D, :], ou[D:D + 1, :])
            rb = sb.tile([P, N], f32, tag="rec2", bufs=2)
            nc.vector.reciprocal_approx_fast(out=rb[0:D, :], in_=raw[0:D, :])
            ot_tmp = sb.tile([P, N], f8, tag="ott", bufs=2)
            # 16/colsum * o_unnorm (the 16 re-centers fp8; undone in g1)
            nc.vector.scalar_tensor_tensor(
                out=ot_tmp[0:D, :], in0=ou[0:D, :], scalar=16.0,
                in1=rb[0:D, :], op0=OP.mult, op1=OP.mult)
            nc.sync.dma_start(out=oT[hh * D:(hh + 1) * D, hp, :],
                              in_=ot_tmp[0:D, :])

        for hp in range(H // 2):
            qh = [qkT[hh * D:(hh + 1) * D, hp, :] for hh in range(2)]
            kh = [qkT[hh * D:(hh + 1) * D, 6 + hp, :] for hh in range(2)]
            o_ps = [[pp.tile([D + 16, HALF], f32, tag="acc", bufs=4,
                             name=f"o_ps{hh}{nk}") for nk in range(2)]
                    for hh in range(2)]
            p8 = [sb.tile([P, 2, N], f8, tag="p8", bufs=4,
                          name=f"p8_{hh}") for hh in range(2)]
            for mt in range(NT):
                if hp == 0:
                    v_tile(mt)
                if hp < H // 2 - 1:
                    if mt == 4:
                        qk_block(hp + 1)
                    elif mt == 6:
                        qk_block(hp + 7)
                for hh in range(2):
                    h = 2 * hp + hh
                    rpb_t = sb.tile([P, N], bf16, tag="rpb", bufs=6)
                    nc.sync.dma_start(out=rpb_t,
                                      in_=rpb_d[h, mt * P:(mt + 1) * P, :])
                    st = pp.tile([P, N], f32, tag="st", bufs=2, name="st")
                    for nk in range(2):
                        nc.tensor.matmul(st[:, NSL[nk]],
                                         lhsT=kh[hh][:, mt * P:(mt + 1) * P],
                                         rhs=qh[hh][:, NSL[nk]],
                                         start=True, stop=True)
                    # P~ = exp(s/512)*exp(rpb): scalar reads PSUM with its
                    # free affine scale; DVE does an all-16-bit multiply
                    texp = sb.tile([P, N], bf16, tag="texp", bufs=4)
                    nc.scalar.activation(out=texp, in_=st, func=AF.Exp,
                                         scale=1.0 / 512.0)
                    pslot = p8[hh][:, mt % 2, :]
                    if has_mask:
                        nc.vector.scalar_tensor_tensor(
                            out=pslot, in0=texp, scalar=maskb[:, mt:mt + 1],
                            in1=rpb_t, op0=OP.mult, op1=OP.mult)
                    elif mt in (0, 2, 4):
                        # offload 6/16 of the P=exp(s)*exp(rpb) multiplies per
                        # head-pair to the otherwise-idle GpSimd engine; only
                        # even-mt tiles (their AV consumer is a full mul-time
                        # away, so GpSimd's ~2.4us latency stays off-path)
                        nc.gpsimd.tensor_tensor(out=pslot, in0=texp,
                                                in1=rpb_t, op=OP.mult)
                    else:
                        nc.vector.tensor_mul(pslot, texp, rpb_t)
                if mt % 2 == 1:
                    for hh in range(2):
                        for nk in range(2):
                            nc.tensor.matmul(
                                o_ps[hh][nk][:, :],
                                lhsT=vaug[:, mt // 2, 2 * hp + hh, :, :],
                                rhs=p8[hh][:, :, NSL[nk]],
                                start=(mt == 1), stop=(mt == NT - 1),
                                perf_mode=PM.DoubleRow)
                    if mt < NT - 1:
                        p8 = [sb.tile([P, 2, N], f8, tag="p8", bufs=4,
                                      name=f"p8_{hh}") for hh in range(2)]
            evac_head(o_ps[0], hp, 0)
            evac_head(o_ps[1], hp, 1)

        # ---------------- proj + residual 1 ----------------
        for mt in range(KC):
            wt = sb.tile([P, KC, P], f8, tag="w6", bufs=3)
            nc.sync.dma_start(out=wt, in_=pw_d[mt])
            for nk in range(2):
                mm = pp.tile([P, HALF], f32, tag="acc", bufs=4)
                for kp in range(KC // 2):
                    nc.tensor.matmul(mm, lhsT=wt[:, 2 * kp:2 * kp + 2],
                                     rhs=oT[:, 2 * kp:2 * kp + 2, NSL[nk]],
                                     start=(kp == 0), stop=(kp == KC // 2 - 1),
                                     perf_mode=PM.DoubleRow)
                if pb_nz:
                    nc.vector.tensor_scalar_add(mm, mm, vec(V_PB)[:, mt:mt + 1])
                nc.vector.scalar_tensor_tensor(
                    out=xres[:, mt, NSL[nk]], in0=mm,
                    scalar=vec(V_G1)[:, mt:mt + 1],
                    in1=xres[:, mt, NSL[nk]], op0=OP.mult, op1=OP.add)

        # ---------------- LN2, adapter-down, MLP ----------------
        xn2T = sb.tile([P, KC, N], f8, tag="feat", bufs=2)
        layernorm(xn2T, V_L2G, V_L2B, ln2_triv)

        a1T = sb.tile([P, KR, N], f8, tag="a1", bufs=1)
        for mt in range(KR):
            wt = sb.tile([P, KC, P], f8, tag="w6", bufs=3)
            nc.sync.dma_start(out=wt, in_=ad_d[mt])
            for nk in range(2):
                mm = pp.tile([P, HALF], f32, tag="acc", bufs=4)
                for kp in range(KC // 2):
                    nc.tensor.matmul(mm, lhsT=wt[:, 2 * kp:2 * kp + 2],
                                     rhs=xn2T[:, 2 * kp:2 * kp + 2, NSL[nk]],
                                     start=(kp == 0), stop=(kp == KC // 2 - 1),
                                     perf_mode=PM.DoubleRow)
                nc.scalar.activation(
                    out=a1T[:, mt, NSL[nk]], in_=mm, func=AF.Relu,
                    scale=1.0 / WS,
                    bias=(adb[:, mt:mt + 1] if adb_nz else zero_col))

        # fc1 for BOTH token halves interleaved: 12 DoubleRow matmuls per
        # gelu-pair window keeps the PE dense (max p-state); each fc1 weight
        # tile is loaded once. Two output blocks share one 2-bank PSUM tile
        # so a single [128,1024] gelu covers them.
        h1s = [sb.tile([P, KM, HALF], f8, tag=f"h1_{nk}", bufs=1,
                       name=f"h1_{nk}") for nk in range(2)]
        for mtp in range(KM // 2):
            wts = []
            for j in range(2):
                wt = sb.tile([P, KC, P], f8, tag="w6", bufs=3)
                nc.sync.dma_start(out=wt, in_=f1_d[2 * mtp + j])
                wts.append(wt)
            for nk in range(2):
                mm2 = pp.tile([P, N], f32, tag="st", bufs=2)
                for j in range(2):
                    for kp in range(KC // 2):
                        nc.tensor.matmul(mm2[:, j * HALF:(j + 1) * HALF],
                                         lhsT=wts[j][:, 2 * kp:2 * kp + 2],
                                         rhs=xn2T[:, 2 * kp:2 * kp + 2, NSL[nk]],
                                         start=(kp == 0),
                                         stop=(kp == KC // 2 - 1),
                                         perf_mode=PM.DoubleRow)
                h1 = h1s[nk]
                if f1b_nz:
                    for j in range(2):
                        mt = 2 * mtp + j
                        nc.scalar.activation(
                            out=h1[:, mt], in_=mm2[:, j * HALF:(j + 1) * HALF],
                            func=AF.Gelu, scale=1.0 / WS,
                            bias=f1b[:, mt:mt + 1])
                else:
                    nc.scalar.activation(
                        out=h1[:, 2 * mtp:2 * mtp + 2].rearrange(
                            "p a t -> p (a t)"),
                        in_=mm2, func=AF.Gelu, scale=1.0 / WS, bias=zero_col)
        for mt in range(KC):
            w2 = sb.tile([P, KM, P], f8, tag="w24", bufs=3)
            nc.sync.dma_start(out=w2, in_=f2_d[mt])
            au = sb.tile([P, KR, P], f8, tag="w2", bufs=2)
            nc.sync.dma_start(out=au, in_=au_d[mt])
            for nk in range(2):
                mm = pp.tile([P, HALF], f32, tag="acc", bufs=4)
                for kp in range(KM // 2):
                    nc.tensor.matmul(mm, lhsT=w2[:, 2 * kp:2 * kp + 2],
                                     rhs=h1s[nk][:, 2 * kp:2 * kp + 2],
                                     start=(kp == 0), stop=False,
                                     perf_mode=PM.DoubleRow)
                nc.tensor.matmul(mm, lhsT=au, rhs=a1T[:, :, NSL[nk]],
                                 start=False, stop=True,
                                 perf_mode=PM.DoubleRow)
                if fb_nz:
                    nc.vector.tensor_scalar_add(mm, mm, vec(V_FB)[:, mt:mt + 1])
                nc.vector.scalar_tensor_tensor(
                    out=xres[:, mt, NSL[nk]], in0=mm,
                    scalar=vec(V_G2)[:, mt:mt + 1],
                    in1=xres[:, mt, NSL[nk]], op0=OP.mult, op1=OP.add)
                nc.sync.dma_start(out=out_d[:, mt, NSL[nk]],
                                  in_=xres[:, mt, NSL[nk]])

    if not nc.is_finalized():
        nc.finalize()
    return nc


def _pack_w6(wT, km, kk):
    """[K, M] (K=contraction, M=out) -> [M//128, 128, K//128, 128] tiles
    laid out so each DMA partition read is contiguous."""
    K, M = wT.shape
    assert K == kk * P and M == km * P
    a = wT.reshape(kk, P, km, P)          # [ks, p, mt, col]
    return np.ascontiguousarray(a.transpose(2, 1, 0, 3)).astype(F8)


def _stripe(v, k):
    """[k*128] -> [128, k] with v[ks*128+p] at [p, ks]."""
    return np.ascontiguousarray(v.reshape(k, P).T).astype(np.float32)


def prepare_core_inputs(x, mask, rpb, ln1_g, ln1_b, qkv_w, q_bias, v_bias,
                        proj_w, proj_b, gamma1, ln2_g, ln2_b, fc1_w, fc1_b,
                        fc2_w, fc2_b, ad_dw, ad_db, ad_uw, ad_ub, gamma2):
    """Host-side layout prep. Returns (per_core_maps, flags)."""
    f32 = np.float32

    qkv_w = np.asarray(qkv_w, f32)
    wq = qkv_w[:C] * WS            # both x8; with 1/sqrt(D)=1/8 the scores
    wk = qkv_w[C:2 * C] * WS       # need a 1/(8*8*8)=1/512 softmax-side scale
    wv = qkv_w[2 * C:] * WS
    wqkT = np.concatenate([wq, wk], 0).T          # [C, 1536]
    wqk = _pack_w6(wqkT, 12, KC)
    # wv used as matmul rhs: [p, ks, col] = wv[col, ks*128+p]
    wv_packed = np.ascontiguousarray(
        wv.T.reshape(KC, P, C).transpose(1, 0, 2)).astype(F8)

    projw = _pack_w6(np.asarray(proj_w, f32).T * WS, KC, KC)
    fc1w = _pack_w6(np.asarray(fc1_w, f32).T * WS, KM, KC)
    fc2w = _pack_w6(np.asarray(fc2_w, f32).T * WS, KC, KM)
    adw = _pack_w6(np.asarray(ad_dw, f32).T * WS, KR, KC)
    auw = _pack_w6(np.asarray(ad_uw, f32).T * WS, KC, KR)

    # exp(rpb) so the kernel folds the bias into softmax as a multiply
    rpbT = np.ascontiguousarray(
        np.exp(np.asarray(rpb, f32).transpose(0, 2, 1))).astype(BF16)

    fb = (np.asarray(fc2_b, f32) + np.asarray(ad_ub, f32)) * WS

    # gamma1 compensates: v x8, evac x16, projw x8  -> /1024
    # gamma2 compensates: fc2w/auw x8 -> /8; fb prescaled x8 to match
    vecs = np.stack([
        _stripe(np.asarray(gamma1, f32) / (WS * 16.0 * WS), KC),
        _stripe(np.asarray(gamma2, f32) / WS, KC),
        _stripe(np.asarray(q_bias, f32) * WS, KC),
        _stripe(np.asarray(proj_b, f32) * (WS * 16.0 * WS), KC),
        _stripe(fb, KC),
        _stripe(np.asarray(ln1_g, f32), KC),
        _stripe(np.asarray(ln1_b, f32), KC),
        _stripe(np.asarray(ln2_g, f32), KC),
        _stripe(np.asarray(ln2_b, f32), KC),
    ], 0)  # [NVEC, 128, KC]

    f1b = _stripe(np.asarray(fc1_b, f32), KM)
    adb = _stripe(np.asarray(ad_db, f32), KR)
    vb = (np.asarray(v_bias, f32) * WS).reshape(1, C).astype(f32)

    mask = np.asarray(mask)
    has_mask = not bool(mask.all())

    flags = (
        has_mask,
        bool(np.any(np.asarray(q_bias, f32))),
        bool(np.any(v_bias)),
        bool(np.any(proj_b)),
        bool(np.any(fc1_b)),
        bool(np.any(fb)),
        bool(np.any(ad_db)),
        bool(np.all(ln1_g == 1.0) and not np.any(ln1_b)),
        bool(np.all(ln2_g == 1.0) and not np.any(ln2_b)),
    )

    shared = {
        "rpbT": rpbT, "wqk": wqk, "wv": wv_packed, "projw": projw,
        "fc1w": fc1w, "fc2w": fc2w, "adw": adw, "auw": auw,
        "vecs": vecs, "fc1b": f1b, "adb": adb, "vbias": vb,
    }

    x = np.asarray(x, f32)
    per_core = []
    for b in range(B):
        xT = np.ascontiguousarray(
            x[b].T.reshape(KC, P, N).transpose(1, 0, 2)).astype(f32)
        if has_mask:
            mb = np.where(mask[b], 1.0, 0.0).astype(f32)    # [N] over keys m
            mb = np.ascontiguousarray(mb.reshape(NT, P).T)  # [128, NT]
        else:
            mb = np.zeros((P, NT), f32)
        m = dict(shared)
        m["xT"] = xT
        m["maskb"] = mb
        per_core.append(m)
    return per_core, flags


def _ensure_ntff_hook():
    """The agent image lacks ``antenv.axon_hooks``; provide it and register
    the ctypes NTFF profile hook so trace=True works under axon."""
    import types
    try:
        from antenv.axon_hooks import get_axon_ntff_profile_hook  # noqa: F401
        return
    except ImportError:
        pass
    import antenv
    mod = types.ModuleType("antenv.axon_hooks")
    _h = {"hook": None}
    mod.set_axon_ntff_profile_hook = lambda h: _h.__setitem__("hook", h)
    mod.get_axon_ntff_profile_hook = lambda: _h["hook"]
    sys.modules["antenv.axon_hooks"] = mod
    antenv.axon_hooks = mod
    try:
        from trn_agent_boot.trn_boot import _ntff_profile_via_ctypes
        hook = _ntff_profile_via_ctypes("/opt/axon/libaxon_pjrt.so")
        if hook is not None:
            mod.set_axon_ntff_profile_hook(hook)
    except Exception as e:  # profiling degrades, run still works
        print("ntff hook setup failed:", e)


def run_sharded(inputs, trace=False, trace_kwargs=None):
    """Compile (cached) + run on 8 cores. Returns (out [B,N,C] f32, results)."""
    from concourse.bass_utils import run_bass_kernel_spmd
    if trace:
        _ensure_ntff_hook()

    per_core, flags = prepare_core_inputs(**inputs)
    if flags not in _PROG_CACHE:
        _PROG_CACHE[flags] = _build(flags)
    nc = _PROG_CACHE[flags]

    kw = {}
    if trace:
        kw["trace"] = True
        kw["trace_cores"] = [0]
        if trace_kwargs:
            kw["trace_kwargs"] = trace_kwargs
    res = run_bass_kernel_spmd(nc, per_core, core_ids=list(range(B)), **kw)

    out = np.empty((B, N, C), np.float32)
    for b in range(B):
        oT = res.results[b]["outT"]          # [128, KC, N]
        out[b] = oT.transpose(1, 0, 2).reshape(C, N).T
    return out, res


def kernel(**inputs):
    out, _ = run_sharded(inputs, trace=False)
    return out
